# revision 11
# baseline (speedup 1.0000x reference)
"""CIN (Compressed Interaction Network) forward pass on 8 Trainium2 cores.

Reference computation (per sample b, per embedding dim d — fully pointwise
in (b, d)):
    x0 = inputs[b, :, d]                    # [40]
    h  = x0
    per layer i (W_i maps Fi*40 -> 256):
        z  = outer(h, x0).flatten()         # [Fi * 40], index f*40+g
        y  = relu(z @ W_i + b_i)            # [256]
        outputs_i = y[:128] (i<3) else y    # pooled
        h  = y[128:]                        # fields fed forward
    out[b] = sum_d concat(outputs)          # [512]

Sharding: data-parallel over batch (64 samples/core), weights replicated.

Per-core kernel strategy:
  - Everything is blocked over bd = (b*32 + d) in chunks of 512 columns.
  - Layout: all tensors column-major in bd ("transposed"): h^T [fields, bd].
  - z^T tiles ([K, 512], K rows on partitions = contraction dim) are built
    on the Vector engine as elementwise products: z^T[(f,g), :] =
    h^T[f, :] * x0^T[g, :].  The x0 row needed on all 128 partitions is
    materialized once per chunk by a single DMA with a stride-0 partition
    source (x0bcast[p, g, :] = x0^T[g, :] for every p).
  - GEMM on the tensor engine in fp16 (1 cycle/row; fp32 is 4x slower),
    accumulating fp32 in PSUM over the 40 (L2/3) or 14 (L1) k-tiles.
  - Scalar engine applies bias+ReLU straight out of PSUM, casting to fp16
    (h for the next layer / y-halves for pooling).
  - Vector engine reduces y-halves over d (groups of 32) into the output
    accumulator; final DMA writes [512 fields, 64 batch] per core.
  - Layer order is software-pipelined L1(0), [L2(c), L1(c+1), L3(c)] so the
    tensor engine never waits on the h hand-off between layers.
"""

import numpy as np

import concourse.bass as bass
import concourse.tile as tile
from concourse import bacc, mybir
from concourse.bass import ds

F32 = mybir.dt.float32
F16 = mybir.dt.float16

B, F0, D = 512, 40, 32
N_CORES = 8
B_CORE = B // N_CORES            # 64
BD = B_CORE * D                  # 2048
CHUNK = 512
N_CHUNKS = BD // CHUNK           # 4
B_CHUNK = CHUNK // D             # 16 batch rows per chunk
FI = 128                         # h fields for layers 2/3
NOUT = 256
L1_TK = 120                      # L1 k-tile = 3 f-values x 40 g-values
L1_FULL_TILES = 13               # 13*120 = 1560 rows
L1_LAST_K = 40                   # + 1 tile of 40 rows (f=39)
RELU = mybir.ActivationFunctionType.Relu

_BUILD_CACHE = {}


def _build(reps=1, trace_sim=False):
    """Build + schedule + bacc-compile the per-core program."""
    nc = bacc.Bacc("TRN2", target_bir_lowering=False, debug=False,
                   num_devices=N_CORES)

    x0t = nc.declare_dram_parameter("x0t", [N_CHUNKS, F0, CHUNK], F16, isOutput=False)
    w1a = nc.declare_dram_parameter("w1a", [L1_TK, L1_FULL_TILES, NOUT], F16, isOutput=False)
    w1b = nc.declare_dram_parameter("w1b", [L1_LAST_K, 1, NOUT], F16, isOutput=False)
    w2 = nc.declare_dram_parameter("w2", [FI, F0, NOUT], F16, isOutput=False)
    w3 = nc.declare_dram_parameter("w3", [FI, F0, NOUT], F16, isOutput=False)
    b1 = nc.declare_dram_parameter("b1", [NOUT], F32, isOutput=False)
    b2 = nc.declare_dram_parameter("b2", [NOUT], F32, isOutput=False)
    b3 = nc.declare_dram_parameter("b3", [NOUT], F32, isOutput=False)
    out = nc.declare_dram_parameter("out", [4 * FI, B_CORE], F32, isOutput=True)

    with tile.TileContext(nc, trace_sim=trace_sim) as tc:
        import contextlib
        with contextlib.ExitStack() as ctx:
            wpool = ctx.enter_context(tc.tile_pool(name="w", bufs=1))
            opool = ctx.enter_context(tc.tile_pool(name="o", bufs=1))
            x0bpool = ctx.enter_context(tc.tile_pool(name="x0b", bufs=2))
            l1pool = ctx.enter_context(tc.tile_pool(name="l1", bufs=2))
            zpool = ctx.enter_context(tc.tile_pool(name="z", bufs=10))
            hpool = ctx.enter_context(tc.tile_pool(name="h", bufs=2))
            ypool = ctx.enter_context(tc.tile_pool(name="y", bufs=3))
            pspool = ctx.enter_context(tc.tile_pool(name="ps", bufs=6, space="PSUM"))

            # ---- resident constants ----
            w1a_sb = wpool.tile([L1_TK, L1_FULL_TILES, NOUT], F16, tag="w1a")
            nc.scalar.dma_start(out=w1a_sb[:], in_=w1a[:])
            w1b_sb = wpool.tile([L1_LAST_K, 1, NOUT], F16, tag="w1b")
            nc.scalar.dma_start(out=w1b_sb[:], in_=w1b[:])
            w2_sb = wpool.tile([FI, F0, NOUT], F16, tag="w2")
            nc.sync.dma_start(out=w2_sb[:], in_=w2[:])
            w3_sb = wpool.tile([FI, F0, NOUT], F16, tag="w3")
            nc.sync.dma_start(out=w3_sb[:], in_=w3[:])
            bias = {}
            for nm, t in (("b1", b1), ("b2", b2), ("b3", b3)):
                for half in range(2):
                    bt = wpool.tile([FI, 1], F32, tag=f"{nm}_{half}", name=f"{nm}_{half}")
                    nc.scalar.dma_start(out=bt[:], in_=t[ds(half * FI, FI)].unsqueeze(1))
                    bias[(nm, half)] = bt
            # output accumulators [fields(128) x batch], one per field block
            oacc = [opool.tile([FI, B_CORE], F32, tag=f"oacc{k}", name=f"oacc{k}")
                    for k in range(4)]

            def act_pool_half(ps_half, bias_ap, oidx, c):
                """relu+bias+sum-over-d fused on ACT: 16 ops of [128, 32],
                each writing one batch column of the output accumulator."""
                ysc = ypool.tile([FI, CHUNK], F16, tag="y", name=f"ysc_{oidx}_{c}")
                for bb in range(B_CHUNK):
                    nc.scalar.activation(
                        ysc[:, ds(bb * D, D)], ps_half[:, ds(bb * D, D)],
                        RELU, bias=bias_ap,
                        accum_out=oacc[oidx][:, ds(c * B_CHUNK + bb, 1)])

            h_tiles = {}

            def emit_x0b(c):
                # x0 broadcast tile for chunk c (used by its L2/L3 z-builds):
                # stride-0 partition source -> every partition holds x0^T chunk.
                # Split in 4 so consumers can start on the first quarter.
                if c >= N_CHUNKS or ("x0b", c) in h_tiles:
                    return
                x0b = x0bpool.tile([128, F0, CHUNK], F16, tag="x0b")
                for q in range(4):
                    eng = nc.sync if q % 2 == 0 else nc.scalar
                    eng.dma_start(
                        out=x0b[:, ds(q * 10, 10), :],
                        in_=x0t[c, ds(q * 10, 10), :].partition_broadcast(128))
                h_tiles[("x0b", c)] = x0b

            def emit_l1(c):

                ps = [pspool.tile([FI, CHUNK], F32, tag="ps", name=f"ps1_{c}_{i}")
                      for i in range(2)]
                # supertiles: in0su[p, t, :] = x0^T[3t + p//40], in1su[p, t, :]
                # = x0^T[p%40] — all 13 full k-tiles in 2 DMAs.
                # source order must be (f, rep, t, j) to match dest (p, t, j).
                in0su = l1pool.tile([L1_TK, L1_FULL_TILES, CHUNK], F16, tag="l1in0")
                in1r = l1pool.tile([L1_TK, CHUNK], F16, tag="l1in1")
                for cf in range(3):
                    # in0su[40cf + r, t, :] = x0^T[3t + cf]  (r = 0..39)
                    nc.sync.dma_start(
                        out=in0su[ds(40 * cf, F0), :, :],
                        in_=x0t[c, cf:39:3, :].unsqueeze(0)
                            .broadcast_to((F0, L1_FULL_TILES, CHUNK)))
                    # in1r[40cf + r, :] = x0^T[r] — same rows for every k-tile
                    nc.scalar.dma_start(out=in1r[ds(40 * cf, F0), :], in_=x0t[c])
                in0l = l1pool.tile([L1_LAST_K, CHUNK], F16, tag="l1in0l")
                nc.sync.dma_start(
                    out=in0l[:], in_=x0t[c, F0 - 1:F0, :].partition_broadcast(L1_LAST_K))
                in1l = l1pool.tile([L1_LAST_K, CHUNK], F16, tag="l1in1l")
                nc.scalar.dma_start(out=in1l[:], in_=x0t[c])
                for t in range(L1_FULL_TILES + 1):
                    if t < L1_FULL_TILES:
                        kk, lhs = L1_TK, w1a_sb[:, t, :]
                        i0, i1 = in0su[:, t, :], in1r[:]
                    else:
                        kk, lhs = L1_LAST_K, w1b_sb[:, 0, :]
                        i0, i1 = in0l[:], in1l[:]
                    z = l1pool.tile([kk, CHUNK], F16, tag="l1z", bufs=8)
                    nc.vector.tensor_mul(z[:], i0, i1)
                    for n in range(2):
                        nc.tensor.matmul(ps[n][:], lhsT=lhs[:, ds(n * FI, FI)],
                                         rhs=z[:], start=(t == 0),
                                         stop=(t == L1_FULL_TILES))
                # n0 half -> pooled output block 0 ; n1 half -> h1
                act_pool_half(ps[0], bias[("b1", 0)][:], 0, c)
                h1 = hpool.tile([FI, CHUNK], F16, tag="h1")
                nc.scalar.activation(h1[:], ps[1][:], RELU, bias=bias[("b1", 1)][:])
                h_tiles[("h1", c)] = h1

            def emit_l23(c, layer):
                w_sb = w2_sb if layer == 2 else w3_sb
                bnm = "b2" if layer == 2 else "b3"
                hin = h_tiles[("h1", c)] if layer == 2 else h_tiles[("h2", c)]
                x0b = h_tiles[("x0b", c)]
                ps = [pspool.tile([FI, CHUNK], F32, tag="ps", name=f"ps{layer}_{c}_{i}")
                      for i in range(2)]
                for g in range(F0):
                    z = zpool.tile([FI, CHUNK], F16, tag="z")
                    nc.vector.tensor_mul(z[:], hin[:], x0b[:, g, :])
                    for n in range(2):
                        nc.tensor.matmul(ps[n][:], lhsT=w_sb[:, g, ds(n * FI, FI)],
                                         rhs=z[:], start=(g == 0), stop=(g == F0 - 1))
                if layer == 2:
                    act_pool_half(ps[0], bias[(bnm, 0)][:], 1, c)
                    h2 = hpool.tile([FI, CHUNK], F16, tag="h2")
                    nc.scalar.activation(h2[:], ps[1][:], RELU, bias=bias[(bnm, 1)][:])
                    h_tiles[("h2", c)] = h2
                else:
                    for n in range(2):
                        act_pool_half(ps[n], bias[(bnm, n)][:], 2 + n, c)

            def emit_body():
                emit_x0b(0)
                emit_l1(0)
                for c in range(N_CHUNKS):
                    emit_x0b(c + 1)
                    emit_l23(c, 2)
                    if c + 1 < N_CHUNKS:
                        emit_l1(c + 1)
                    emit_l23(c, 3)
                for k in range(4):
                    nc.sync.dma_start(out=out[ds(k * FI, FI), :], in_=oacc[k][:])

            if reps == 1:
                emit_body()
            else:
                with tc.For_i(0, reps, 1):
                    emit_body()

    nc.compile()
    return nc


def _get_nc(reps=1):
    if reps not in _BUILD_CACHE:
        _BUILD_CACHE[reps] = _build(reps)
    return _BUILD_CACHE[reps]


def _prep_inputs(inputs, W1, b1, W2, b2, W3, b3):
    """Host-side shard + layout prep (cheap reshapes/casts only)."""
    f16 = np.float16
    w1a = np.ascontiguousarray(
        W1[:1560].reshape(L1_FULL_TILES, L1_TK, NOUT).transpose(1, 0, 2)).astype(f16)
    w1b = np.ascontiguousarray(W1[1560:].reshape(1, L1_LAST_K, NOUT)
                               .transpose(1, 0, 2)).astype(f16)
    w2h = np.ascontiguousarray(W2.reshape(FI, F0, NOUT)).astype(f16)
    w3h = np.ascontiguousarray(W3.reshape(FI, F0, NOUT)).astype(f16)
    b1f = np.ascontiguousarray(b1, dtype=np.float32)
    b2f = np.ascontiguousarray(b2, dtype=np.float32)
    b3f = np.ascontiguousarray(b3, dtype=np.float32)
    in_maps = []
    for core in range(N_CORES):
        xc = inputs[core * B_CORE:(core + 1) * B_CORE]          # [64, 40, 32]
        t = xc.transpose(1, 0, 2).reshape(F0, BD)                # [40, 2048]
        x0tc = np.ascontiguousarray(
            t.reshape(F0, N_CHUNKS, CHUNK).transpose(1, 0, 2)).astype(f16)
        in_maps.append({
            "x0t": x0tc, "w1a": w1a, "w1b": w1b, "w2": w2h, "w3": w3h,
            "b1": b1f, "b2": b2f, "b3": b3f,
        })
    return in_maps


def _unshard(results):
    # per-core out: [512 fields, 64 local batch] -> [B, 512]
    full = np.concatenate([r["out"] for r in results], axis=1)   # [512, 512]
    return np.ascontiguousarray(full.T)


def kernel(inputs, W1, b1, W2, b2, W3, b3):
    from concourse.bass_utils import run_bass_kernel_spmd
    nc = _get_nc(reps=1)
    in_maps = _prep_inputs(inputs, W1, b1, W2, b2, W3, b3)
    res = run_bass_kernel_spmd(nc, in_maps, list(range(N_CORES)))
    return _unshard(res.results)
